# revision 17
# baseline (speedup 1.0000x reference)
"""Multi-head attention Trainium2 kernel (8 NeuronCores).

Sharding: core c handles batch b=c//4 and head group g=c%4 (4 of 16 heads).
Fully "transposed" formulation (no on-device transposes):
  qT/kT [dq, s] via lhsT=W-pair, rhs=X^T;  v [s, dk] via lhsT=X^T-chunk, rhs=Wv
  scoresT[s_k, s_q] via lhsT=kT-chunk, rhs=qT (softmax axis = partition dim)
  exp fused on ScalarE (scale=1/sqrt(dq)); rowsum via ones-columns in the
  attn@v matmul; oT[dk, s_q] is exactly the lhsT the output projection wants.
An 8-wide AllToAll (each quarter sent to shards j and j+4 so both batches'
rank j receive it) reshards from (4 local heads, all s) to (all 16 heads,
s-quarter); each core loads only its batch's half of cc_out via a dynamic
(partition_id-derived) DMA offset, computes its final [512, 1024] output
slice, and the host concatenates.

The attention inner loop is ScalarE(exp)-bound (~1.15us per [128,1024]
exp tile); all other PE work (pair-1 projections, pair-0 half of the
output projection) is emitted as 4-matmul micro-batches between attention
iterations so the in-order PE queue can absorb it in the ACT slack.
"""

import sys

if "/opt/trn_rl_repo" not in sys.path:
    sys.path.insert(0, "/opt/trn_rl_repo")

import numpy as np
import ml_dtypes

import concourse.bass as bass
import concourse.bacc as bacc
import concourse.bass_utils as bass_utils
import concourse.mybir as mybir
import concourse.tile as tile
from concourse.tile_rust import add_dep_helper

# pad attention iterations with dead matmuls to keep PE duty near 100% so
# the HAM clock-gate holds the PE at 2.4 GHz through the exp-bound phase
WARM_PAD = True

B, S, DIN = 2, 2048, 1024
H, DK = 16, 64
NCORES = 8
HL = 4  # heads per core
SQ = S // 4  # output rows per core

F32 = mybir.dt.float32
BF16 = mybir.dt.bfloat16
BF16NP = ml_dtypes.bfloat16

DC = DIN // 128  # 8 din chunks
SKC = S // 128  # 16 s_k chunks
VW = 2 * DK  # 128: 64 v columns + 64 ones columns (rowsum broadcast via PE)
NR = 4  # ranks per batch group


def build():
    nc = bacc.Bacc("TRN2", target_bir_lowering=False, debug=False, num_devices=NCORES)

    xqt = nc.dram_tensor("xqt", [DIN, S], BF16, kind="ExternalInput")
    xkt = nc.dram_tensor("xkt", [DIN, S], BF16, kind="ExternalInput")
    xvt = nc.dram_tensor("xvt", [DIN, S], BF16, kind="ExternalInput")
    wq = nc.dram_tensor("wq", [DIN, HL * DK], BF16, kind="ExternalInput")
    wk = nc.dram_tensor("wk", [DIN, HL * DK], BF16, kind="ExternalInput")
    wv = nc.dram_tensor("wv", [DIN, HL * DK], BF16, kind="ExternalInput")
    # packed wo: rows 512*p + 128*j + 64*hh = Wo[64*(4j+2p+hh)]
    wo = nc.dram_tensor("wo", [H * DK, DIN], BF16, kind="ExternalInput")
    bqp = nc.dram_tensor("bqp", [128, 2], F32, kind="ExternalInput")
    bkp = nc.dram_tensor("bkp", [128, 2], F32, kind="ExternalInput")
    bvr = nc.dram_tensor("bvr", [128, HL * DK], F32, kind="ExternalInput")
    bor = nc.dram_tensor("bor", [128, DIN], F32, kind="ExternalInput")
    out = nc.dram_tensor("out", [SQ, DIN], F32, kind="ExternalOutput")

    with tile.TileContext(nc) as tc:
        with (
            tc.tile_pool(name="pers", bufs=1) as pers,
            tc.tile_pool(name="work", bufs=3) as work,
            tc.tile_pool(name="wrk2", bufs=2) as wrk2,
            tc.tile_pool(name="psmm", bufs=2, space="PSUM") as psmm,
            tc.tile_pool(name="psacc", bufs=1, space="PSUM") as psacc,
            tc.tile_pool(name="pspj", bufs=1, space="PSUM") as pspj,
            tc.tile_pool(name="psjk", bufs=1, space="PSUM") as psjk,
            tc.tile_pool(name="dram", bufs=1, space="DRAM") as dram,
        ):
            # ---- weights / biases (small, on sync queue) ----
            wq_sb = pers.tile([128, DC, HL * DK], BF16)
            wk_sb = pers.tile([128, DC, HL * DK], BF16)
            wv_sb = pers.tile([128, DC, HL * DK], BF16)
            nc.sync.dma_start(wk_sb[:], wk.rearrange("(c p) d -> p c d", p=128))
            nc.sync.dma_start(wv_sb[:], wv.rearrange("(c p) d -> p c d", p=128))
            nc.sync.dma_start(wq_sb[:], wq.rearrange("(c p) d -> p c d", p=128))
            bq_sb = pers.tile([128, 2], F32)
            bk_sb = pers.tile([128, 2], F32)
            bv_sb = pers.tile([128, HL * DK], F32)
            bo_sb = pers.tile([128, DIN], F32)
            nc.sync.dma_start(bk_sb[:], bkp[:])
            nc.sync.dma_start(bq_sb[:], bqp[:])
            nc.sync.dma_start(bv_sb[:], bvr[:])
            nc.sync.dma_start(bo_sb[:], bor[:])

            # ---- X^T loads: xk then xv then xq, streamed per s-block ----
            xq_sb = pers.tile([128, DC, S], BF16, tag="big", bufs=3, name="xq_sb")
            xk_sb = pers.tile([128, DC, S], BF16, tag="big", bufs=3, name="xk_sb")
            xv_sb = pers.tile([128, DC, S], BF16, tag="big", bufs=3, name="xv_sb")
            for xsb, xdram in ((xk_sb, xkt), (xv_sb, xvt), (xq_sb, xqt)):
                for sblk in range(4):
                    ssl = slice(512 * sblk, 512 * (sblk + 1))
                    nc.gpsimd.dma_start(
                        xsb[:, :, ssl],
                        xdram[:, ssl].rearrange("(c p) s -> p c s", p=128),
                    )

            qt_sb = [pers.tile([128, S], BF16, name=f"qt{p}") for p in range(2)]
            kt_sb = [pers.tile([128, S], BF16, name=f"kt{p}") for p in range(2)]
            v_sb = pers.tile([128, SKC, HL * VW], BF16)

            # Filler machinery: proj / oproj-partial matmul work is queued as
            # small closures and drained between attention iterations so the
            # in-order PE queue interleaves it into the exp-bound stream.
            pending = []

            def drain_fillers(n=1):
                for _ in range(n):
                    if pending:
                        pending.pop(0)()

            def emit_qk_halves(which, p, sb, defer):
                xsb, wsb, bsb, dst = {
                    "q": (xq_sb, wq_sb, bq_sb, qt_sb),
                    "k": (xk_sb, wk_sb, bk_sb, kt_sb),
                }[which]
                state = {}

                def half(h):
                    def go():
                        if h == 0:
                            state["ps"] = pspj.tile([128, 512], F32, tag="pj", name="psqk")
                        ps = state["ps"]
                        for c in range(4 * h, 4 * h + 4):
                            nc.tensor.matmul(
                                ps[:],
                                wsb[:, c, 128 * p : 128 * (p + 1)],
                                xsb[:, c, 512 * sb : 512 * (sb + 1)],
                                start=(c == 0),
                                stop=(c == DC - 1),
                            )
                        if h == 1:
                            nc.vector.tensor_scalar_add(
                                dst[p][:, 512 * sb : 512 * (sb + 1)],
                                ps[:],
                                bsb[:, p : p + 1],
                            )
                    return go

                if defer:
                    pending.append(half(0))
                    pending.append(half(1))
                else:
                    half(0)()
                    half(1)()

            def emit_v_group(sc):
                # v projection, all 4 heads in one pass (N=256, 8 matmuls)
                psv = pspj.tile([128, HL * DK], F32, tag="pj", name="psv")
                for c in range(DC):
                    nc.tensor.matmul(
                        psv[:],
                        xv_sb[:, c, 128 * sc : 128 * (sc + 1)],
                        wv_sb[:, c, :],
                        start=(c == 0),
                        stop=(c == DC - 1),
                    )
                for h in range(HL):
                    nc.vector.tensor_add(
                        v_sb[:, sc, h * VW : h * VW + DK],
                        psv[:, h * DK : (h + 1) * DK],
                        bv_sb[:, h * DK : (h + 1) * DK],
                    )

            # ---- collective staging ----
            cc_in = [dram.tile([8 * 2 * DK, SQ], BF16, name=f"cc_in{p}") for p in range(2)]
            cc_out = [dram.tile([8 * 2 * DK, SQ], BF16, name=f"cc_out{p}") for p in range(2)]

            def emit_a2a(p):
                nc.gpsimd.collective_compute(
                    "AllToAll",
                    mybir.AluOpType.bypass,
                    replica_groups=[[0, 1, 2, 3, 4, 5, 6, 7]],
                    ins=[cc_in[p].opt()],
                    outs=[cc_out[p].opt()],
                )

            last_ot = [None]
            junk_tile = [None]

            def emit_junk(n):
                if not WARM_PAD:
                    return
                if junk_tile[0] is None:
                    junk_tile[0] = psjk.tile([128, 512], F32, tag="jk", name="junk")
                for _ in range(n):
                    nc.tensor.matmul(
                        junk_tile[0][0:32, :],
                        kt_sb[0][0:64, 0:32],
                        kt_sb[0][0:64, 0:512],
                        start=True,
                        stop=True,
                    )

            gate_inst = [None]

            def emit_attention_sqb(p, sqb, fill_every=0, record_gate=False):
                qsl = slice(512 * sqb, 512 * (sqb + 1))
                po = psacc.tile([128, 1024], F32, tag="acc", name="po")
                for skc in range(SKC):
                    ps2 = psmm.tile([128, 1024], F32, tag="mm", name="ps2")
                    for ch in range(2):
                        cs = slice(64 * ch, 64 * (ch + 1))
                        mm = nc.tensor.matmul(
                            ps2[:, 512 * ch : 512 * (ch + 1)],
                            kt_sb[p][cs, 128 * skc : 128 * (skc + 1)],
                            qt_sb[p][cs, qsl],
                            start=True,
                            stop=True,
                        )
                        if record_gate and skc == 0 and ch == 0:
                            gate_inst[0] = mm.ins
                    et = work.tile([128, 1024], BF16, tag="et", name="et")
                    nc.scalar.activation(
                        et[:],
                        ps2[:],
                        mybir.ActivationFunctionType.Exp,
                        bias=0.0,
                        scale=float(1.0 / np.sqrt(DK)),
                    )
                    filled = False
                    if fill_every and pending and skc % fill_every == 0:
                        drain_fillers(1)
                        filled = True
                    emit_junk(1 if filled else 2)
                    for ch in range(2):
                        h = 2 * p + ch
                        nc.tensor.matmul(
                            po[:, 512 * ch : 512 * (ch + 1)],
                            v_sb[:, skc, h * VW : h * VW + VW],
                            et[:, 512 * ch : 512 * (ch + 1)],
                            start=(skc == 0),
                            stop=(skc == SKC - 1),
                        )
                # epilogue: rowsum reciprocal, normalize, stage for the a2a
                rcp = wrk2.tile([128, 1024], F32, tag="rcp", name="rcp")
                rlo = wrk2.tile([64, 1024], F32, tag="rlo", name="rlo")
                ot = wrk2.tile([64, 1024], BF16, tag="ot", name="ot")
                nc.vector.reciprocal_approx_fast(out=rcp[:], in_=po[:])
                nc.sync.dma_start(rlo[:], rcp[64:128, :])
                nc.vector.tensor_mul(ot[:], po[0:DK, :], rlo[:])
                last_ot[0] = ot
                for shard in (sqb, sqb + 4):
                    for ch in range(2):
                        base = shard * 2 * DK + ch * DK
                        nc.sync.dma_start(
                            cc_in[p][base : base + DK, :],
                            ot[:, 512 * ch : 512 * (ch + 1)],
                        )

            # ---- schedule ----
            for h in range(HL):
                nc.vector.memset(v_sb[:, :, h * VW + DK : (h + 1) * VW], 1.0)
            # pair-0 prerequisites, tracking the xk -> xv -> xq load order
            for sb in range(4):
                emit_qk_halves("k", 0, sb, defer=False)
            for sc in range(SKC):
                emit_v_group(sc)
            for sb in range(4):
                emit_qk_halves("q", 0, sb, defer=False)
            # pair-1 projections become filler inside attention pair 0
            for sb in range(4):
                emit_qk_halves("k", 1, sb, defer=True)
            for sb in range(4):
                emit_qk_halves("q", 1, sb, defer=True)

            for sqb in range(4):
                emit_attention_sqb(0, sqb, fill_every=4)
            drain_fillers(len(pending))
            emit_a2a(0)
            # wo reuses xq's slot (dead after qt proj), ol reuses xk's slot
            wo_sb = pers.tile([128, 2 * NR, DIN], BF16, tag="big", bufs=3, name="wo_sb")
            nc.gpsimd.dma_start(wo_sb[:], wo.rearrange("(c p) d -> p c d", p=128))
            ol_sb = pers.tile([128, 2 * NR, SQ], BF16, tag="big", bufs=3, name="ol_sb")
            # load only this batch's half of cc_out (senders 4b..4b+3) via a
            # dynamic source offset: batch = partition_id // 4
            pid = nc.gpsimd.partition_id()
            boff = (pid // 4) * (512 * SQ)

            def emit_ol(p):
                half0 = cc_out[p][0:512, :].rearrange("(c p) s -> p c s", p=128)
                src = bass.AP(
                    tensor=half0.tensor,
                    offset=half0.offset + boff,
                    ap=half0.ap,
                    dep_tracking_offset=0,
                )
                nc.gpsimd.dma_start(ol_sb[:, NR * p : NR * (p + 1), :], src)

            emit_ol(0)

            # pair-0 half of the output projection runs as filler inside the
            # last attention-pair-1 quarters (pso over chunks 0..3 -> SBUF,
            # bias pre-added so the tail is one tensor_add per tile)
            part_sb = pers.tile([128, 8, 512], F32, name="part_sb")

            def emit_opart(sb2, do):
                def go():
                    pso = pspj.tile([128, 512], F32, tag="pj", name="psop")
                    first = True
                    for c in range(NR):
                        mm = nc.tensor.matmul(
                            pso[:],
                            ol_sb[:, c, 128 * sb2 : 128 * (sb2 + 1)],
                            wo_sb[:, c, 512 * do : 512 * (do + 1)],
                            start=(c == 0),
                            stop=(c == NR - 1),
                        )
                        if first and gate_inst[0] is not None:
                            # pin the pair-0 output-projection partials behind
                            # the last attention quarter so the scheduler can't
                            # hoist them ahead of the AllToAll's completion
                            add_dep_helper(
                                mm.ins, gate_inst[0], sync=True, reason="opart gate"
                            )
                            first = False
                    nc.vector.tensor_add(
                        part_sb[:, 2 * sb2 + do, :],
                        pso[:],
                        bo_sb[:, 512 * do : 512 * (do + 1)],
                    )
                return go

            emit_attention_sqb(1, 0)
            emit_attention_sqb(1, 1)
            emit_attention_sqb(1, 2)
            for sb2 in range(4):
                for do in range(2):
                    pending.append(emit_opart(sb2, do))
            emit_attention_sqb(1, 3, fill_every=2, record_gate=True)
            drain_fillers(len(pending))
            emit_a2a(1)
            emit_ol(1)

            # warm-keeper matmuls: keep the PE busy (and HAM un-throttled)
            # while the second AllToAll is on the wire; results are unused.
            ot = last_ot[0]
            junk = psmm.tile([128, 512], F32, tag="mm", name="junk")
            for r in range(48):
                nc.tensor.matmul(
                    junk[:],
                    ot[:, 0:128],
                    ot[:, 0:512],
                    start=True,
                    stop=True,
                )

            # ---- output projection tail: pair-1 chunks + stored partials ----
            for sb2 in range(4):
                os_sb = wrk2.tile([128, DIN], F32, tag="os", name="os")
                for do in range(2):
                    pso = psmm.tile([128, 512], F32, tag="mm", name="pso")
                    for c in range(NR, 2 * NR):
                        nc.tensor.matmul(
                            pso[:],
                            ol_sb[:, c, 128 * sb2 : 128 * (sb2 + 1)],
                            wo_sb[:, c, 512 * do : 512 * (do + 1)],
                            start=(c == NR),
                            stop=(c == 2 * NR - 1),
                        )
                    nc.vector.tensor_add(
                        os_sb[:, 512 * do : 512 * (do + 1)],
                        pso[:],
                        part_sb[:, 2 * sb2 + do, :],
                    )
                nc.sync.dma_start(out[128 * sb2 : 128 * (sb2 + 1), :], os_sb[:])

    nc.compile()
    return nc


_NC = None


def _get_nc():
    global _NC
    if _NC is None:
        _NC = build()
    return _NC


def _pack_wo(Wo):
    """Row order matches ol_sb chunks: c = 4p + j (pair p, sender rank-in-group
    j whose head group is j); within a chunk, 64 rows per head hh."""
    out = np.zeros((H * DK, DIN), np.float32)
    for p in range(2):
        for j in range(NR):
            for hh in range(2):
                hg = 4 * j + 2 * p + hh
                dst = 512 * p + 128 * j + 64 * hh
                out[dst : dst + 64, :] = Wo[hg * 64 : (hg + 1) * 64, :]
    return out


def make_in_maps(Q, K, V, Wq, bq, Wk, bk, Wv, bv, Wo, bo):
    Q, K, V = (np.asarray(a, np.float32) for a in (Q, K, V))
    Wq, bq, Wk, bk, Wv, bv = (
        np.asarray(a, np.float32) for a in (Wq, bq, Wk, bk, Wv, bv)
    )
    Wo = np.asarray(Wo, np.float32)
    bo = np.asarray(bo, np.float32)
    wo_packed = np.ascontiguousarray(_pack_wo(Wo).astype(BF16NP))
    bo_b = np.ascontiguousarray(np.broadcast_to(bo, (128, DIN)))
    xts = []
    for b in range(B):
        xts.append(
            tuple(np.ascontiguousarray(A[b].T.astype(BF16NP)) for A in (Q, K, V))
        )
    in_maps = []
    for c in range(NCORES):
        b, g = divmod(c, 4)
        hs = slice(HL * g, HL * (g + 1))
        bq2 = np.ascontiguousarray(bq[hs].reshape(2, 128).T)
        bk2 = np.ascontiguousarray(bk[hs].reshape(2, 128).T)
        xq_t, xk_t, xv_t = xts[b]
        in_maps.append(
            {
                "xqt": xq_t,
                "xkt": xk_t,
                "xvt": xv_t,
                "wq": np.ascontiguousarray(
                    Wq[hs].transpose(1, 0, 2).reshape(DIN, HL * DK).astype(BF16NP)
                ),
                "wk": np.ascontiguousarray(
                    Wk[hs].transpose(1, 0, 2).reshape(DIN, HL * DK).astype(BF16NP)
                ),
                "wv": np.ascontiguousarray(
                    Wv[hs].transpose(1, 0, 2).reshape(DIN, HL * DK).astype(BF16NP)
                ),
                "wo": wo_packed,
                "bqp": bq2,
                "bkp": bk2,
                "bvr": np.ascontiguousarray(
                    np.broadcast_to(bv[hs].reshape(-1), (128, HL * DK))
                ),
                "bor": bo_b,
            }
        )
    return in_maps


def run(nc, in_maps, **kwargs):
    return bass_utils.run_bass_kernel_spmd(
        nc, in_maps, core_ids=list(range(NCORES)), **kwargs
    )


def kernel(Q, K, V, Wq, bq, Wk, bk, Wv, bv, Wo, bo):
    nc = _get_nc()
    in_maps = make_in_maps(Q, K, V, Wq, bq, Wk, bk, Wv, bv, Wo, bo)
    res = run(nc, in_maps)
    full = np.empty((B, S, DIN), np.float32)
    for c in range(NCORES):
        b, g = divmod(c, 4)
        full[b, SQ * g : SQ * (g + 1), :] = res.results[c]["out"]
    return full


# revision 20
# speedup vs baseline: 1.0636x; 1.0636x over previous
"""Multi-head attention Trainium2 kernel (8 NeuronCores).

Sharding: core c handles batch b=c//4 and head group g=c%4 (4 of 16 heads).
Fully "transposed" formulation (no on-device transposes):
  qT/kT [dq, s] via lhsT=W-pair, rhs=X^T;  v [s, dk] via lhsT=X^T-chunk, rhs=Wv
  scoresT[s_k, s_q] via lhsT=kT-chunk, rhs=qT (softmax axis = partition dim)
  exp fused on ScalarE (scale=1/sqrt(dq)); rowsum via ones-columns in the
  attn@v matmul; oT[dk, s_q] is exactly the lhsT the output projection wants.
An 8-wide AllToAll (each quarter sent to shards j and j+4 so both batches'
rank j receive it) reshards from (4 local heads, all s) to (all 16 heads,
s-quarter); each core loads only its batch's half of cc_out via a dynamic
(partition_id-derived) DMA offset, computes its final [512, 1024] output
slice, and the host concatenates.

The attention inner loop is ScalarE(exp)-bound (~1.15us per [128,1024]
exp tile); all other PE work (pair-1 projections, pair-0 half of the
output projection) is emitted as 4-matmul micro-batches between attention
iterations so the in-order PE queue can absorb it in the ACT slack.
"""

import sys

if "/opt/trn_rl_repo" not in sys.path:
    sys.path.insert(0, "/opt/trn_rl_repo")

import numpy as np
import ml_dtypes

import concourse.bass as bass
import concourse.bacc as bacc
import concourse.bass_utils as bass_utils
import concourse.mybir as mybir
import concourse.tile as tile
from concourse.tile_rust import add_dep_helper

# pad attention iterations with dead matmuls to keep PE duty near 100%:
# measured ineffective (PE stays clock-capped regardless of duty), so off
WARM_PAD = False

B, S, DIN = 2, 2048, 1024
H, DK = 16, 64
NCORES = 8
HL = 4  # heads per core
SQ = S // 4  # output rows per core

F32 = mybir.dt.float32
BF16 = mybir.dt.bfloat16
BF16NP = ml_dtypes.bfloat16

DC = DIN // 128  # 8 din chunks
SKC = S // 128  # 16 s_k chunks
VW = 2 * DK  # 128: 64 v columns + 64 ones columns (rowsum broadcast via PE)
NR = 4  # ranks per batch group


def build():
    nc = bacc.Bacc("TRN2", target_bir_lowering=False, debug=False, num_devices=NCORES)

    xqt = nc.dram_tensor("xqt", [DIN, S], BF16, kind="ExternalInput")
    xkt = nc.dram_tensor("xkt", [DIN, S], BF16, kind="ExternalInput")
    xvt = nc.dram_tensor("xvt", [DIN, S], BF16, kind="ExternalInput")
    wq = nc.dram_tensor("wq", [DIN, HL * DK], BF16, kind="ExternalInput")
    wk = nc.dram_tensor("wk", [DIN, HL * DK], BF16, kind="ExternalInput")
    wv = nc.dram_tensor("wv", [DIN, HL * DK], BF16, kind="ExternalInput")
    # packed wo: rows 512*p + 128*j + 64*hh = Wo[64*(4j+2p+hh)]
    wo = nc.dram_tensor("wo", [H * DK, DIN], BF16, kind="ExternalInput")
    bqp = nc.dram_tensor("bqp", [128, 2], F32, kind="ExternalInput")
    bkp = nc.dram_tensor("bkp", [128, 2], F32, kind="ExternalInput")
    bvr = nc.dram_tensor("bvr", [128, HL * DK], F32, kind="ExternalInput")
    bor = nc.dram_tensor("bor", [128, DIN], F32, kind="ExternalInput")
    out = nc.dram_tensor("out", [SQ, DIN], F32, kind="ExternalOutput")

    with tile.TileContext(nc) as tc:
        with (
            tc.tile_pool(name="pers", bufs=1) as pers,
            tc.tile_pool(name="work", bufs=3) as work,
            tc.tile_pool(name="wrk2", bufs=2) as wrk2,
            tc.tile_pool(name="psmm", bufs=2, space="PSUM") as psmm,
            tc.tile_pool(name="psacc", bufs=1, space="PSUM") as psacc,
            tc.tile_pool(name="pspj", bufs=2, space="PSUM") as pspj,
            tc.tile_pool(name="dram", bufs=1, space="DRAM") as dram,
        ):
            # ---- weights / biases (small, on sync queue) ----
            wq_sb = pers.tile([128, DC, HL * DK], BF16)
            wk_sb = pers.tile([128, DC, HL * DK], BF16)
            wv_sb = pers.tile([128, DC, HL * DK], BF16)
            nc.sync.dma_start(wk_sb[:], wk.rearrange("(c p) d -> p c d", p=128))
            nc.sync.dma_start(wv_sb[:], wv.rearrange("(c p) d -> p c d", p=128))
            nc.sync.dma_start(wq_sb[:], wq.rearrange("(c p) d -> p c d", p=128))
            bq_sb = pers.tile([128, 2], F32)
            bk_sb = pers.tile([128, 2], F32)
            bv_sb = pers.tile([128, HL * DK], F32)
            bo_sb = pers.tile([128, DIN], F32)
            nc.sync.dma_start(bk_sb[:], bkp[:])
            nc.sync.dma_start(bq_sb[:], bqp[:])
            nc.sync.dma_start(bv_sb[:], bvr[:])
            nc.sync.dma_start(bo_sb[:], bor[:])

            # ---- X^T loads: xk then xv then xq, streamed per s-block ----
            xq_sb = pers.tile([128, DC, S], BF16, tag="big", bufs=3, name="xq_sb")
            xk_sb = pers.tile([128, DC, S], BF16, tag="big", bufs=3, name="xk_sb")
            xv_sb = pers.tile([128, DC, S], BF16, tag="big", bufs=3, name="xv_sb")
            for xsb, xdram in ((xk_sb, xkt), (xv_sb, xvt), (xq_sb, xqt)):
                for sblk in range(4):
                    ssl = slice(512 * sblk, 512 * (sblk + 1))
                    nc.gpsimd.dma_start(
                        xsb[:, :, ssl],
                        xdram[:, ssl].rearrange("(c p) s -> p c s", p=128),
                    )

            qt_sb = [pers.tile([128, S], BF16, name=f"qt{p}") for p in range(2)]
            kt_sb = [pers.tile([128, S], BF16, name=f"kt{p}") for p in range(2)]
            v_sb = pers.tile([128, SKC, HL * VW], BF16)

            # Filler machinery: proj / oproj-partial matmul work is queued as
            # small closures and drained between attention iterations so the
            # in-order PE queue interleaves it into the exp-bound stream.
            pending = []

            def drain_fillers(n=1):
                for _ in range(n):
                    if pending:
                        pending.pop(0)()

            def emit_qk_halves(which, p, sb, defer):
                xsb, wsb, bsb, dst = {
                    "q": (xq_sb, wq_sb, bq_sb, qt_sb),
                    "k": (xk_sb, wk_sb, bk_sb, kt_sb),
                }[which]
                state = {}

                def half(h):
                    def go():
                        if h == 0:
                            state["ps"] = pspj.tile([128, 512], F32, tag="pj", name="psqk")
                        ps = state["ps"]
                        for c in range(4 * h, 4 * h + 4):
                            nc.tensor.matmul(
                                ps[:],
                                wsb[:, c, 128 * p : 128 * (p + 1)],
                                xsb[:, c, 512 * sb : 512 * (sb + 1)],
                                start=(c == 0),
                                stop=(c == DC - 1),
                            )
                        if h == 1:
                            nc.vector.tensor_scalar_add(
                                dst[p][:, 512 * sb : 512 * (sb + 1)],
                                ps[:],
                                bsb[:, p : p + 1],
                            )
                    return go

                if defer:
                    pending.append(half(0))
                    pending.append(half(1))
                else:
                    half(0)()
                    half(1)()

            def emit_v_group(sc):
                # v projection, all 4 heads in one pass (N=256, 8 matmuls)
                psv = pspj.tile([128, HL * DK], F32, tag="pj", name="psv")
                for c in range(DC):
                    nc.tensor.matmul(
                        psv[:],
                        xv_sb[:, c, 128 * sc : 128 * (sc + 1)],
                        wv_sb[:, c, :],
                        start=(c == 0),
                        stop=(c == DC - 1),
                    )
                for h in range(HL):
                    nc.vector.tensor_add(
                        v_sb[:, sc, h * VW : h * VW + DK],
                        psv[:, h * DK : (h + 1) * DK],
                        bv_sb[:, h * DK : (h + 1) * DK],
                    )

            # ---- collective staging ----
            cc_in = [dram.tile([8 * 2 * DK, SQ], BF16, name=f"cc_in{p}") for p in range(2)]
            cc_out = [dram.tile([8 * 2 * DK, SQ], BF16, name=f"cc_out{p}") for p in range(2)]

            def emit_a2a(p):
                nc.gpsimd.collective_compute(
                    "AllToAll",
                    mybir.AluOpType.bypass,
                    replica_groups=[[0, 1, 2, 3, 4, 5, 6, 7]],
                    ins=[cc_in[p].opt()],
                    outs=[cc_out[p].opt()],
                )

            last_ot = [None]
            junk_tile = [None]

            def emit_junk(n):
                if not WARM_PAD:
                    return

            gate_inst = [None]

            def emit_attention_sqb(p, sqb, fill_every=0, record_gate=False):
                qsl = slice(512 * sqb, 512 * (sqb + 1))
                po = psacc.tile([128, 1024], F32, tag="acc", name="po")
                for skc in range(SKC):
                    ps2 = psmm.tile([128, 1024], F32, tag="mm", name="ps2")
                    for ch in range(2):
                        cs = slice(64 * ch, 64 * (ch + 1))
                        mm = nc.tensor.matmul(
                            ps2[:, 512 * ch : 512 * (ch + 1)],
                            kt_sb[p][cs, 128 * skc : 128 * (skc + 1)],
                            qt_sb[p][cs, qsl],
                            start=True,
                            stop=True,
                        )
                        if record_gate and skc == 0 and ch == 0:
                            gate_inst[0] = mm.ins
                    et = work.tile([128, 1024], BF16, tag="et", name="et")
                    nc.scalar.activation(
                        et[:],
                        ps2[:],
                        mybir.ActivationFunctionType.Exp,
                        bias=0.0,
                        scale=float(1.0 / np.sqrt(DK)),
                    )
                    filled = False
                    if fill_every and pending and skc % fill_every == 0:
                        drain_fillers(1)
                        filled = True
                    emit_junk(1 if filled else 2)
                    for ch in range(2):
                        h = 2 * p + ch
                        nc.tensor.matmul(
                            po[:, 512 * ch : 512 * (ch + 1)],
                            v_sb[:, skc, h * VW : h * VW + VW],
                            et[:, 512 * ch : 512 * (ch + 1)],
                            start=(skc == 0),
                            stop=(skc == SKC - 1),
                        )
                # epilogue: rowsum reciprocal, normalize, stage for the a2a
                rcp = wrk2.tile([128, 1024], F32, tag="rcp", name="rcp")
                rlo = wrk2.tile([64, 1024], F32, tag="rlo", name="rlo")
                ot = wrk2.tile([64, 1024], BF16, tag="ot", name="ot")
                nc.vector.reciprocal_approx_fast(out=rcp[:], in_=po[:])
                nc.sync.dma_start(rlo[:], rcp[64:128, :])
                nc.vector.tensor_mul(ot[:], po[0:DK, :], rlo[:])
                last_ot[0] = ot
                for shard in (sqb, sqb + 4):
                    for ch in range(2):
                        base = shard * 2 * DK + ch * DK
                        nc.sync.dma_start(
                            cc_in[p][base : base + DK, :],
                            ot[:, 512 * ch : 512 * (ch + 1)],
                        )

            # ---- schedule ----
            for h in range(HL):
                nc.vector.memset(v_sb[:, :, h * VW + DK : (h + 1) * VW], 1.0)
            # pair-0 prerequisites, tracking the xk -> xv -> xq load order
            for sb in range(4):
                emit_qk_halves("k", 0, sb, defer=False)
            for sc in range(SKC):
                emit_v_group(sc)
            for sb in range(4):
                emit_qk_halves("q", 0, sb, defer=False)
            # pair-1 projections become filler inside attention pair 0
            for sb in range(4):
                emit_qk_halves("k", 1, sb, defer=True)
            for sb in range(4):
                emit_qk_halves("q", 1, sb, defer=True)

            for sqb in range(4):
                emit_attention_sqb(0, sqb, fill_every=4)
            drain_fillers(len(pending))
            emit_a2a(0)
            # wo reuses xq's slot (dead after qt proj), ol reuses xk's slot
            wo_sb = pers.tile([128, 2 * NR, DIN], BF16, tag="big", bufs=3, name="wo_sb")
            nc.gpsimd.dma_start(wo_sb[:], wo.rearrange("(c p) d -> p c d", p=128))
            ol_sb = pers.tile([128, 2 * NR, SQ], BF16, tag="big", bufs=3, name="ol_sb")
            # load only this batch's half of cc_out (senders 4b..4b+3) via a
            # dynamic source offset: batch = partition_id // 4
            pid = nc.gpsimd.partition_id()
            boff = (pid // 4) * (512 * SQ)

            def emit_ol(p):
                half0 = cc_out[p][0:512, :].rearrange("(c p) s -> p c s", p=128)
                src = bass.AP(
                    tensor=half0.tensor,
                    offset=half0.offset + boff,
                    ap=half0.ap,
                    dep_tracking_offset=0,
                )
                nc.gpsimd.dma_start(ol_sb[:, NR * p : NR * (p + 1), :], src)

            emit_ol(0)

            # pair-0 half of the output projection runs as filler inside the
            # last attention-pair-1 quarters (pso over chunks 0..3 -> SBUF,
            # bias pre-added so the tail is one tensor_add per tile)
            part_sb = pers.tile([128, 8, 512], F32, name="part_sb")

            def emit_opart(sb2, do):
                def go():
                    pso = pspj.tile([128, 512], F32, tag="pj", name="psop")
                    first = True
                    for c in range(NR):
                        mm = nc.tensor.matmul(
                            pso[:],
                            ol_sb[:, c, 128 * sb2 : 128 * (sb2 + 1)],
                            wo_sb[:, c, 512 * do : 512 * (do + 1)],
                            start=(c == 0),
                            stop=(c == NR - 1),
                        )
                        if first and gate_inst[0] is not None:
                            # pin the pair-0 output-projection partials behind
                            # the last attention quarter so the scheduler can't
                            # hoist them ahead of the AllToAll's completion
                            add_dep_helper(
                                mm.ins, gate_inst[0], sync=True, reason="opart gate"
                            )
                            first = False
                    nc.vector.tensor_add(
                        part_sb[:, 2 * sb2 + do, :],
                        pso[:],
                        bo_sb[:, 512 * do : 512 * (do + 1)],
                    )
                return go

            emit_attention_sqb(1, 0)
            emit_attention_sqb(1, 1)
            emit_attention_sqb(1, 2)
            for sb2 in range(4):
                for do in range(2):
                    pending.append(emit_opart(sb2, do))
            emit_attention_sqb(1, 3, fill_every=2, record_gate=True)
            drain_fillers(len(pending))
            emit_a2a(1)
            emit_ol(1)

            # warm-keeper matmuls: keep the PE busy (and HAM un-throttled)
            # while the second AllToAll is on the wire; results are unused.
            ot = last_ot[0]
            junk = psmm.tile([128, 512], F32, tag="mm", name="junk")
            for r in range(48):
                nc.tensor.matmul(
                    junk[:],
                    ot[:, 0:128],
                    ot[:, 0:512],
                    start=True,
                    stop=True,
                )

            # ---- output projection tail: pair-1 chunks + stored partials ----
            for sb2 in range(4):
                os_sb = wrk2.tile([128, DIN], F32, tag="os", name="os")
                for do in range(2):
                    pso = psmm.tile([128, 512], F32, tag="mm", name="pso")
                    for c in range(NR, 2 * NR):
                        nc.tensor.matmul(
                            pso[:],
                            ol_sb[:, c, 128 * sb2 : 128 * (sb2 + 1)],
                            wo_sb[:, c, 512 * do : 512 * (do + 1)],
                            start=(c == NR),
                            stop=(c == 2 * NR - 1),
                        )
                    nc.vector.tensor_add(
                        os_sb[:, 512 * do : 512 * (do + 1)],
                        pso[:],
                        part_sb[:, 2 * sb2 + do, :],
                    )
                nc.sync.dma_start(out[128 * sb2 : 128 * (sb2 + 1), :], os_sb[:])

    nc.compile()
    return nc


_NC = None


def _get_nc():
    global _NC
    if _NC is None:
        _NC = build()
    return _NC


def _pack_wo(Wo):
    """Row order matches ol_sb chunks: c = 4p + j (pair p, sender rank-in-group
    j whose head group is j); within a chunk, 64 rows per head hh."""
    out = np.zeros((H * DK, DIN), np.float32)
    for p in range(2):
        for j in range(NR):
            for hh in range(2):
                hg = 4 * j + 2 * p + hh
                dst = 512 * p + 128 * j + 64 * hh
                out[dst : dst + 64, :] = Wo[hg * 64 : (hg + 1) * 64, :]
    return out


def make_in_maps(Q, K, V, Wq, bq, Wk, bk, Wv, bv, Wo, bo):
    Q, K, V = (np.asarray(a, np.float32) for a in (Q, K, V))
    Wq, bq, Wk, bk, Wv, bv = (
        np.asarray(a, np.float32) for a in (Wq, bq, Wk, bk, Wv, bv)
    )
    Wo = np.asarray(Wo, np.float32)
    bo = np.asarray(bo, np.float32)
    wo_packed = np.ascontiguousarray(_pack_wo(Wo).astype(BF16NP))
    bo_b = np.ascontiguousarray(np.broadcast_to(bo, (128, DIN)))
    xts = []
    for b in range(B):
        xts.append(
            tuple(np.ascontiguousarray(A[b].T.astype(BF16NP)) for A in (Q, K, V))
        )
    in_maps = []
    for c in range(NCORES):
        b, g = divmod(c, 4)
        hs = slice(HL * g, HL * (g + 1))
        bq2 = np.ascontiguousarray(bq[hs].reshape(2, 128).T)
        bk2 = np.ascontiguousarray(bk[hs].reshape(2, 128).T)
        xq_t, xk_t, xv_t = xts[b]
        in_maps.append(
            {
                "xqt": xq_t,
                "xkt": xk_t,
                "xvt": xv_t,
                "wq": np.ascontiguousarray(
                    Wq[hs].transpose(1, 0, 2).reshape(DIN, HL * DK).astype(BF16NP)
                ),
                "wk": np.ascontiguousarray(
                    Wk[hs].transpose(1, 0, 2).reshape(DIN, HL * DK).astype(BF16NP)
                ),
                "wv": np.ascontiguousarray(
                    Wv[hs].transpose(1, 0, 2).reshape(DIN, HL * DK).astype(BF16NP)
                ),
                "wo": wo_packed,
                "bqp": bq2,
                "bkp": bk2,
                "bvr": np.ascontiguousarray(
                    np.broadcast_to(bv[hs].reshape(-1), (128, HL * DK))
                ),
                "bor": bo_b,
            }
        )
    return in_maps


def run(nc, in_maps, **kwargs):
    return bass_utils.run_bass_kernel_spmd(
        nc, in_maps, core_ids=list(range(NCORES)), **kwargs
    )


def kernel(Q, K, V, Wq, bq, Wk, bk, Wv, bv, Wo, bo):
    nc = _get_nc()
    in_maps = make_in_maps(Q, K, V, Wq, bq, Wk, bk, Wv, bv, Wo, bo)
    res = run(nc, in_maps)
    full = np.empty((B, S, DIN), np.float32)
    for c in range(NCORES):
        b, g = divmod(c, 4)
        full[b, SQ * g : SQ * (g + 1), :] = res.results[c]["out"]
    return full


# revision 23
# speedup vs baseline: 1.0876x; 1.0226x over previous
"""Multi-head attention Trainium2 kernel (8 NeuronCores).

Sharding: core c handles batch b=c//4 and head group g=c%4 (4 of 16 heads).
Fully "transposed" formulation (no on-device transposes):
  qT/kT [dq, s] via lhsT=W-pair, rhs=X^T;  v [s, dk] via lhsT=X^T-chunk, rhs=Wv
  scoresT[s_k, s_q] via lhsT=kT-chunk, rhs=qT (softmax axis = partition dim)
  exp fused on ScalarE (scale=1/sqrt(dq)); rowsum via ones-columns in the
  attn@v matmul; oT[dk, s_q] is exactly the lhsT the output projection wants.
An 8-wide AllToAll (each quarter sent to shards j and j+4 so both batches'
rank j receive it) reshards from (4 local heads, all s) to (all 16 heads,
s-quarter); each core loads only its batch's half of cc_out via a dynamic
(partition_id-derived) DMA offset, computes its final [512, 1024] output
slice, and the host concatenates.

The attention inner loop is ScalarE(exp)-bound (~1.15us per [128,1024]
exp tile); all other PE work (pair-1 projections, pair-0 half of the
output projection) is emitted as 4-matmul micro-batches between attention
iterations so the in-order PE queue can absorb it in the ACT slack.
"""

import sys

if "/opt/trn_rl_repo" not in sys.path:
    sys.path.insert(0, "/opt/trn_rl_repo")

import numpy as np
import ml_dtypes

import concourse.bass as bass
import concourse.bacc as bacc
import concourse.bass_utils as bass_utils
import concourse.mybir as mybir
import concourse.tile as tile
from concourse.tile_rust import add_dep_helper

# pad attention iterations with dead matmuls to keep PE duty near 100%:
# measured ineffective (PE stays clock-capped regardless of duty), so off
WARM_PAD = False

B, S, DIN = 2, 2048, 1024
H, DK = 16, 64
NCORES = 8
HL = 4  # heads per core
SQ = S // 4  # output rows per core

F32 = mybir.dt.float32
BF16 = mybir.dt.bfloat16
BF16NP = ml_dtypes.bfloat16

DC = DIN // 128  # 8 din chunks
SKC = S // 128  # 16 s_k chunks
VW = 2 * DK  # 128: 64 v columns + 64 ones columns (rowsum broadcast via PE)
NR = 4  # ranks per batch group


def build():
    nc = bacc.Bacc("TRN2", target_bir_lowering=False, debug=False, num_devices=NCORES)

    xqt = nc.dram_tensor("xqt", [DIN, S], BF16, kind="ExternalInput")
    xkt = nc.dram_tensor("xkt", [DIN, S], BF16, kind="ExternalInput")
    xvt = nc.dram_tensor("xvt", [DIN, S], BF16, kind="ExternalInput")
    wq = nc.dram_tensor("wq", [DIN, HL * DK], BF16, kind="ExternalInput")
    wk = nc.dram_tensor("wk", [DIN, HL * DK], BF16, kind="ExternalInput")
    wv = nc.dram_tensor("wv", [DIN, HL * DK], BF16, kind="ExternalInput")
    # packed wo: rows 512*p + 128*j + 64*hh = Wo[64*(4j+2p+hh)]
    wo = nc.dram_tensor("wo", [H * DK, DIN], BF16, kind="ExternalInput")
    bqp = nc.dram_tensor("bqp", [128, 2], F32, kind="ExternalInput")
    bkp = nc.dram_tensor("bkp", [128, 2], F32, kind="ExternalInput")
    bvr = nc.dram_tensor("bvr", [128, HL * DK], F32, kind="ExternalInput")
    bor = nc.dram_tensor("bor", [128, DIN], F32, kind="ExternalInput")
    out = nc.dram_tensor("out", [SQ, DIN], F32, kind="ExternalOutput")

    with tile.TileContext(nc) as tc:
        with (
            tc.tile_pool(name="pers", bufs=1) as pers,
            tc.tile_pool(name="work", bufs=3) as work,
            tc.tile_pool(name="wrk2", bufs=2) as wrk2,
            tc.tile_pool(name="psmm", bufs=2, space="PSUM") as psmm,
            tc.tile_pool(name="psacc", bufs=1, space="PSUM") as psacc,
            tc.tile_pool(name="pspj", bufs=2, space="PSUM") as pspj,
            tc.tile_pool(name="dram", bufs=1, space="DRAM") as dram,
        ):
            # ---- weights / biases (small, on sync queue) ----
            wq_sb = pers.tile([128, DC, HL * DK], BF16)
            wk_sb = pers.tile([128, DC, HL * DK], BF16)
            wv_sb = pers.tile([128, DC, HL * DK], BF16)
            nc.sync.dma_start(wk_sb[:], wk.rearrange("(c p) d -> p c d", p=128))
            nc.sync.dma_start(wv_sb[:], wv.rearrange("(c p) d -> p c d", p=128))
            nc.sync.dma_start(wq_sb[:], wq.rearrange("(c p) d -> p c d", p=128))
            bq_sb = pers.tile([128, 2], F32)
            bk_sb = pers.tile([128, 2], F32)
            bv_sb = pers.tile([128, HL * DK], F32)
            bo_sb = pers.tile([128, DIN], F32)
            nc.sync.dma_start(bk_sb[:], bkp[:])
            nc.sync.dma_start(bq_sb[:], bqp[:])
            nc.sync.dma_start(bv_sb[:], bvr[:])
            nc.sync.dma_start(bo_sb[:], bor[:])

            # ---- X^T loads: xk then xv then xq, streamed per s-block ----
            xq_sb = pers.tile([128, DC, S], BF16, tag="big", bufs=3, name="xq_sb")
            xk_sb = pers.tile([128, DC, S], BF16, tag="big", bufs=3, name="xk_sb")
            xv_sb = pers.tile([128, DC, S], BF16, tag="big", bufs=3, name="xv_sb")
            for xsb, xdram in ((xv_sb, xvt), (xk_sb, xkt), (xq_sb, xqt)):
                for sblk in range(4):
                    ssl = slice(512 * sblk, 512 * (sblk + 1))
                    nc.gpsimd.dma_start(
                        xsb[:, :, ssl],
                        xdram[:, ssl].rearrange("(c p) s -> p c s", p=128),
                    )

            qt_sb = [pers.tile([128, S], BF16, name=f"qt{p}") for p in range(2)]
            kt_sb = [pers.tile([128, S], BF16, name=f"kt{p}") for p in range(2)]
            v_sb = pers.tile([128, SKC, HL * VW], BF16)

            # Filler machinery: proj / oproj-partial matmul work is queued as
            # small closures and drained between attention iterations so the
            # in-order PE queue interleaves it into the exp-bound stream.
            pending = []

            def drain_fillers(n=1):
                for _ in range(n):
                    if pending:
                        pending.pop(0)()

            def emit_qk_halves(which, p, sb, defer):
                xsb, wsb, bsb, dst = {
                    "q": (xq_sb, wq_sb, bq_sb, qt_sb),
                    "k": (xk_sb, wk_sb, bk_sb, kt_sb),
                }[which]
                state = {}

                def half(h):
                    def go():
                        if h == 0:
                            state["ps"] = pspj.tile([128, 512], F32, tag="pj", name="psqk")
                        ps = state["ps"]
                        for c in range(4 * h, 4 * h + 4):
                            nc.tensor.matmul(
                                ps[:],
                                wsb[:, c, 128 * p : 128 * (p + 1)],
                                xsb[:, c, 512 * sb : 512 * (sb + 1)],
                                start=(c == 0),
                                stop=(c == DC - 1),
                            )
                        if h == 1:
                            nc.vector.tensor_scalar_add(
                                dst[p][:, 512 * sb : 512 * (sb + 1)],
                                ps[:],
                                bsb[:, p : p + 1],
                            )
                    return go

                if defer:
                    pending.append(half(0))
                    pending.append(half(1))
                else:
                    half(0)()
                    half(1)()

            def emit_v_group(sc):
                # v projection, all 4 heads in one pass (N=256, 8 matmuls)
                psv = pspj.tile([128, HL * DK], F32, tag="pj", name="psv")
                for c in range(DC):
                    nc.tensor.matmul(
                        psv[:],
                        xv_sb[:, c, 128 * sc : 128 * (sc + 1)],
                        wv_sb[:, c, :],
                        start=(c == 0),
                        stop=(c == DC - 1),
                    )
                for h in range(HL):
                    nc.vector.tensor_add(
                        v_sb[:, sc, h * VW : h * VW + DK],
                        psv[:, h * DK : (h + 1) * DK],
                        bv_sb[:, h * DK : (h + 1) * DK],
                    )

            # ---- collective staging ----
            cc_in = [dram.tile([8 * 2 * DK, SQ], BF16, name=f"cc_in{p}") for p in range(2)]
            cc_out = [dram.tile([8 * 2 * DK, SQ], BF16, name=f"cc_out{p}") for p in range(2)]

            def emit_a2a(p):
                nc.gpsimd.collective_compute(
                    "AllToAll",
                    mybir.AluOpType.bypass,
                    replica_groups=[[0, 1, 2, 3, 4, 5, 6, 7]],
                    ins=[cc_in[p].opt()],
                    outs=[cc_out[p].opt()],
                )

            last_ot = [None]
            junk_tile = [None]

            def emit_junk(n):
                if not WARM_PAD:
                    return

            gate_inst = [None]

            def emit_attention_sqb(p, sqb, fill_every=0, record_gate=False):
                qsl = slice(512 * sqb, 512 * (sqb + 1))
                po = psacc.tile([128, 1024], F32, tag="acc", name="po")
                for skc in range(SKC):
                    ps2 = psmm.tile([128, 1024], F32, tag="mm", name="ps2")
                    for ch in range(2):
                        cs = slice(64 * ch, 64 * (ch + 1))
                        mm = nc.tensor.matmul(
                            ps2[:, 512 * ch : 512 * (ch + 1)],
                            kt_sb[p][cs, 128 * skc : 128 * (skc + 1)],
                            qt_sb[p][cs, qsl],
                            start=True,
                            stop=True,
                        )
                        if record_gate and skc == 0 and ch == 0:
                            gate_inst[0] = mm.ins
                    et = work.tile([128, 1024], BF16, tag="et", name="et")
                    nc.scalar.activation(
                        et[:],
                        ps2[:],
                        mybir.ActivationFunctionType.Exp,
                        bias=0.0,
                        scale=float(1.0 / np.sqrt(DK)),
                    )
                    filled = False
                    if fill_every and pending and skc % fill_every == 0:
                        drain_fillers(1)
                        filled = True
                    emit_junk(1 if filled else 2)
                    for ch in range(2):
                        h = 2 * p + ch
                        nc.tensor.matmul(
                            po[:, 512 * ch : 512 * (ch + 1)],
                            v_sb[:, skc, h * VW : h * VW + VW],
                            et[:, 512 * ch : 512 * (ch + 1)],
                            start=(skc == 0),
                            stop=(skc == SKC - 1),
                        )
                # epilogue: rowsum reciprocal, normalize, stage for the a2a
                rcp = wrk2.tile([128, 1024], F32, tag="rcp", name="rcp")
                rlo = wrk2.tile([64, 1024], F32, tag="rlo", name="rlo")
                ot = wrk2.tile([64, 1024], BF16, tag="ot", name="ot")
                nc.vector.reciprocal_approx_fast(out=rcp[:], in_=po[:])
                nc.sync.dma_start(rlo[:], rcp[64:128, :])
                nc.vector.tensor_mul(ot[:], po[0:DK, :], rlo[:])
                last_ot[0] = ot
                for shard in (sqb, sqb + 4):
                    for ch in range(2):
                        base = shard * 2 * DK + ch * DK
                        nc.sync.dma_start(
                            cc_in[p][base : base + DK, :],
                            ot[:, 512 * ch : 512 * (ch + 1)],
                        )

            # ---- schedule ----
            for h in range(HL):
                nc.vector.memset(v_sb[:, :, h * VW + DK : (h + 1) * VW], 1.0)
            # pair-0 prerequisites, tracking the xv -> xk -> xq load order
            for sc in range(SKC):
                emit_v_group(sc)
            for sb in range(4):
                emit_qk_halves("k", 0, sb, defer=False)
            emit_qk_halves("q", 0, 0, defer=False)
            # remaining projections become filler inside attention pair 0
            for sb in range(1, 4):
                emit_qk_halves("q", 0, sb, defer=True)
            for sb in range(4):
                emit_qk_halves("k", 1, sb, defer=True)
            for sb in range(4):
                emit_qk_halves("q", 1, sb, defer=True)

            for sqb in range(4):
                emit_attention_sqb(0, sqb, fill_every=3)
            drain_fillers(len(pending))
            emit_a2a(0)
            # wo reuses xq's slot (dead after qt proj), ol reuses xk's slot
            wo_sb = pers.tile([128, 2 * NR, DIN], BF16, tag="big", bufs=3, name="wo_sb")
            nc.gpsimd.dma_start(wo_sb[:], wo.rearrange("(c p) d -> p c d", p=128))
            ol_sb = pers.tile([128, 2 * NR, SQ], BF16, tag="big", bufs=3, name="ol_sb")
            # load only this batch's half of cc_out (senders 4b..4b+3) via a
            # dynamic source offset: batch = partition_id // 4
            pid = nc.gpsimd.partition_id()
            boff = (pid // 4) * (512 * SQ)

            def emit_ol(p):
                half0 = cc_out[p][0:512, :].rearrange("(c p) s -> p c s", p=128)
                src = bass.AP(
                    tensor=half0.tensor,
                    offset=half0.offset + boff,
                    ap=half0.ap,
                    dep_tracking_offset=0,
                )
                nc.gpsimd.dma_start(ol_sb[:, NR * p : NR * (p + 1), :], src)

            emit_ol(0)

            # pair-0 half of the output projection runs as filler inside the
            # last attention-pair-1 quarters (pso over chunks 0..3 -> SBUF,
            # bias pre-added so the tail is one tensor_add per tile)
            part_sb = pers.tile([128, 8, 512], F32, name="part_sb")

            def emit_opart(sb2, do):
                def go():
                    pso = pspj.tile([128, 512], F32, tag="pj", name="psop")
                    first = True
                    for c in range(NR):
                        mm = nc.tensor.matmul(
                            pso[:],
                            ol_sb[:, c, 128 * sb2 : 128 * (sb2 + 1)],
                            wo_sb[:, c, 512 * do : 512 * (do + 1)],
                            start=(c == 0),
                            stop=(c == NR - 1),
                        )
                        if first and gate_inst[0] is not None:
                            # pin the pair-0 output-projection partials behind
                            # the last attention quarter so the scheduler can't
                            # hoist them ahead of the AllToAll's completion
                            add_dep_helper(
                                mm.ins, gate_inst[0], sync=True, reason="opart gate"
                            )
                            first = False
                    nc.vector.tensor_add(
                        part_sb[:, 2 * sb2 + do, :],
                        pso[:],
                        bo_sb[:, 512 * do : 512 * (do + 1)],
                    )
                return go

            emit_attention_sqb(1, 0)
            emit_attention_sqb(1, 1)
            emit_attention_sqb(1, 2)
            for sb2 in range(4):
                for do in range(2):
                    pending.append(emit_opart(sb2, do))
            emit_attention_sqb(1, 3, fill_every=2, record_gate=True)
            drain_fillers(len(pending))
            emit_a2a(1)
            emit_ol(1)

            # warm-keeper matmuls: keep the PE busy (and HAM un-throttled)
            # while the second AllToAll is on the wire; results are unused.
            ot = last_ot[0]
            junk = psmm.tile([128, 512], F32, tag="mm", name="junk")
            for r in range(72):
                nc.tensor.matmul(
                    junk[:],
                    ot[:, 0:128],
                    ot[:, 0:512],
                    start=True,
                    stop=True,
                )

            # ---- output projection tail: pair-1 chunks + stored partials ----
            for sb2 in range(4):
                os_sb = wrk2.tile([128, DIN], F32, tag="os", name="os")
                for do in range(2):
                    pso = psmm.tile([128, 512], F32, tag="mm", name="pso")
                    for c in range(NR, 2 * NR):
                        nc.tensor.matmul(
                            pso[:],
                            ol_sb[:, c, 128 * sb2 : 128 * (sb2 + 1)],
                            wo_sb[:, c, 512 * do : 512 * (do + 1)],
                            start=(c == NR),
                            stop=(c == 2 * NR - 1),
                        )
                    nc.vector.tensor_add(
                        os_sb[:, 512 * do : 512 * (do + 1)],
                        pso[:],
                        part_sb[:, 2 * sb2 + do, :],
                    )
                nc.sync.dma_start(out[128 * sb2 : 128 * (sb2 + 1), :], os_sb[:])

    nc.compile()
    return nc


_NC = None


def _get_nc():
    global _NC
    if _NC is None:
        _NC = build()
    return _NC


def _pack_wo(Wo):
    """Row order matches ol_sb chunks: c = 4p + j (pair p, sender rank-in-group
    j whose head group is j); within a chunk, 64 rows per head hh."""
    out = np.zeros((H * DK, DIN), np.float32)
    for p in range(2):
        for j in range(NR):
            for hh in range(2):
                hg = 4 * j + 2 * p + hh
                dst = 512 * p + 128 * j + 64 * hh
                out[dst : dst + 64, :] = Wo[hg * 64 : (hg + 1) * 64, :]
    return out


def make_in_maps(Q, K, V, Wq, bq, Wk, bk, Wv, bv, Wo, bo):
    Q, K, V = (np.asarray(a, np.float32) for a in (Q, K, V))
    Wq, bq, Wk, bk, Wv, bv = (
        np.asarray(a, np.float32) for a in (Wq, bq, Wk, bk, Wv, bv)
    )
    Wo = np.asarray(Wo, np.float32)
    bo = np.asarray(bo, np.float32)
    wo_packed = np.ascontiguousarray(_pack_wo(Wo).astype(BF16NP))
    bo_b = np.ascontiguousarray(np.broadcast_to(bo, (128, DIN)))
    xts = []
    for b in range(B):
        xts.append(
            tuple(np.ascontiguousarray(A[b].T.astype(BF16NP)) for A in (Q, K, V))
        )
    in_maps = []
    for c in range(NCORES):
        b, g = divmod(c, 4)
        hs = slice(HL * g, HL * (g + 1))
        bq2 = np.ascontiguousarray(bq[hs].reshape(2, 128).T)
        bk2 = np.ascontiguousarray(bk[hs].reshape(2, 128).T)
        xq_t, xk_t, xv_t = xts[b]
        in_maps.append(
            {
                "xqt": xq_t,
                "xkt": xk_t,
                "xvt": xv_t,
                "wq": np.ascontiguousarray(
                    Wq[hs].transpose(1, 0, 2).reshape(DIN, HL * DK).astype(BF16NP)
                ),
                "wk": np.ascontiguousarray(
                    Wk[hs].transpose(1, 0, 2).reshape(DIN, HL * DK).astype(BF16NP)
                ),
                "wv": np.ascontiguousarray(
                    Wv[hs].transpose(1, 0, 2).reshape(DIN, HL * DK).astype(BF16NP)
                ),
                "wo": wo_packed,
                "bqp": bq2,
                "bkp": bk2,
                "bvr": np.ascontiguousarray(
                    np.broadcast_to(bv[hs].reshape(-1), (128, HL * DK))
                ),
                "bor": bo_b,
            }
        )
    return in_maps


def run(nc, in_maps, **kwargs):
    return bass_utils.run_bass_kernel_spmd(
        nc, in_maps, core_ids=list(range(NCORES)), **kwargs
    )


def kernel(Q, K, V, Wq, bq, Wk, bk, Wv, bv, Wo, bo):
    nc = _get_nc()
    in_maps = make_in_maps(Q, K, V, Wq, bq, Wk, bk, Wv, bv, Wo, bo)
    res = run(nc, in_maps)
    full = np.empty((B, S, DIN), np.float32)
    for c in range(NCORES):
        b, g = divmod(c, 4)
        full[b, SQ * g : SQ * (g + 1), :] = res.results[c]["out"]
    return full


# revision 28
# speedup vs baseline: 1.1322x; 1.0410x over previous
"""Multi-head attention Trainium2 kernel (8 NeuronCores).

Sharding: core c handles batch b=c//4 and head group g=c%4 (4 of 16 heads).
Fully "transposed" formulation (no on-device transposes):
  qT/kT [dq, s] via lhsT=W-pair, rhs=X^T;  v [s, dk] via lhsT=X^T-chunk, rhs=Wv
  scoresT[s_k, s_q] via lhsT=kT-chunk, rhs=qT (softmax axis = partition dim)
  exp fused on ScalarE (scale=1/sqrt(dq)); rowsum via ones-columns in the
  attn@v matmul; oT[dk, s_q] is exactly the lhsT the output projection wants.
An 8-wide AllToAll (each quarter sent to shards j and j+4 so both batches'
rank j receive it) reshards from (4 local heads, all s) to (all 16 heads,
s-quarter); each core loads only its batch's half of cc_out via a dynamic
(partition_id-derived) DMA offset, computes its final [512, 1024] output
slice, and the host concatenates.

The attention inner loop is ScalarE(exp)-bound (~1.15us per [128,1024]
exp tile); all other PE work (pair-1 projections, pair-0 half of the
output projection) is emitted as 4-matmul micro-batches between attention
iterations so the in-order PE queue can absorb it in the ACT slack.
"""

import sys

if "/opt/trn_rl_repo" not in sys.path:
    sys.path.insert(0, "/opt/trn_rl_repo")

import numpy as np
import ml_dtypes

import concourse.bass as bass
import concourse.bacc as bacc
import concourse.bass_utils as bass_utils
import concourse.mybir as mybir
import concourse.tile as tile
from concourse.tile_rust import add_dep_helper

# pad attention iterations with dead matmuls to keep PE duty near 100%:
# measured ineffective (PE stays clock-capped regardless of duty), so off
WARM_PAD = False

B, S, DIN = 2, 2048, 1024
H, DK = 16, 64
NCORES = 8
HL = 4  # heads per core
SQ = S // 4  # output rows per core

F32 = mybir.dt.float32
BF16 = mybir.dt.bfloat16
BF16NP = ml_dtypes.bfloat16

DC = DIN // 128  # 8 din chunks
SKC = S // 128  # 16 s_k chunks
VW = 2 * DK  # 128: 64 v columns + 64 ones columns (rowsum broadcast via PE)
NR = 4  # ranks per batch group


def build():
    nc = bacc.Bacc("TRN2", target_bir_lowering=False, debug=False, num_devices=NCORES)

    # X^T pre-swizzled on host to [128*sblk + p, c*512 + t] so each DMA row
    # is one 8KB contiguous run; W pre-swizzled to [p, c*256 + d] (4KB rows)
    xqt = nc.dram_tensor("xqt", [512, 8 * 512], BF16, kind="ExternalInput")
    xkt = nc.dram_tensor("xkt", [512, 8 * 512], BF16, kind="ExternalInput")
    xvt = nc.dram_tensor("xvt", [512, 8 * 512], BF16, kind="ExternalInput")
    wq = nc.dram_tensor("wq", [128, DC * HL * DK], BF16, kind="ExternalInput")
    wk = nc.dram_tensor("wk", [128, DC * HL * DK], BF16, kind="ExternalInput")
    wv = nc.dram_tensor("wv", [128, DC * HL * DK], BF16, kind="ExternalInput")
    # packed wo: rows 512*p + 128*j + 64*hh = Wo[64*(4j+2p+hh)]
    wo = nc.dram_tensor("wo", [H * DK, DIN], BF16, kind="ExternalInput")
    bqp = nc.dram_tensor("bqp", [128, 2], F32, kind="ExternalInput")
    bkp = nc.dram_tensor("bkp", [128, 2], F32, kind="ExternalInput")
    bvr = nc.dram_tensor("bvr", [128, HL * DK], F32, kind="ExternalInput")
    bor = nc.dram_tensor("bor", [128, DIN], F32, kind="ExternalInput")
    out = nc.dram_tensor("out", [SQ, DIN], F32, kind="ExternalOutput")

    with tile.TileContext(nc) as tc:
        with (
            tc.tile_pool(name="pers", bufs=1) as pers,
            tc.tile_pool(name="work", bufs=3) as work,
            tc.tile_pool(name="wrk2", bufs=2) as wrk2,
            tc.tile_pool(name="psmm", bufs=2, space="PSUM") as psmm,
            tc.tile_pool(name="psacc", bufs=1, space="PSUM") as psacc,
            tc.tile_pool(name="pspj", bufs=2, space="PSUM") as pspj,
            tc.tile_pool(name="dram", bufs=1, space="DRAM") as dram,
        ):
            # ---- weights / biases (small, on sync queue) ----
            wq_sb = pers.tile([128, DC, HL * DK], BF16)
            wk_sb = pers.tile([128, DC, HL * DK], BF16)
            wv_sb = pers.tile([128, DC, HL * DK], BF16)
            nc.sync.dma_start(wk_sb[:], wk.rearrange("p (c d) -> p c d", d=HL * DK))
            nc.sync.dma_start(wv_sb[:], wv.rearrange("p (c d) -> p c d", d=HL * DK))
            nc.sync.dma_start(wq_sb[:], wq.rearrange("p (c d) -> p c d", d=HL * DK))
            bq_sb = pers.tile([128, 2], F32)
            bk_sb = pers.tile([128, 2], F32)
            bv_sb = pers.tile([128, HL * DK], F32)
            bo_sb = pers.tile([128, DIN], F32)
            nc.sync.dma_start(bk_sb[:], bkp[:])
            nc.sync.dma_start(bq_sb[:], bqp[:])
            nc.sync.dma_start(bv_sb[:], bvr[:])
            nc.sync.dma_start(bo_sb[:], bor[:])

            # ---- X^T loads: xk then xv then xq, streamed per s-block ----
            xq_sb = pers.tile([128, DC, S], BF16, tag="big", bufs=3, name="xq_sb")
            xk_sb = pers.tile([128, DC, S], BF16, tag="big", bufs=3, name="xk_sb")
            xv_sb = pers.tile([128, DC, S], BF16, tag="big", bufs=3, name="xv_sb")
            for xsb, xdram in ((xv_sb, xvt), (xk_sb, xkt), (xq_sb, xqt)):
                for sblk in range(4):
                    ssl = slice(512 * sblk, 512 * (sblk + 1))
                    nc.gpsimd.dma_start(
                        xsb[:, :, ssl],
                        xdram[128 * sblk : 128 * (sblk + 1), :].rearrange(
                            "p (c s) -> p c s", s=512
                        ),
                    )

            qt_sb = [pers.tile([128, S], BF16, name=f"qt{p}") for p in range(2)]
            kt_sb = [pers.tile([128, S], BF16, name=f"kt{p}") for p in range(2)]
            v_sb = pers.tile([128, SKC, HL * VW], BF16)

            # Filler machinery: proj / oproj-partial matmul work is queued as
            # small closures and drained between attention iterations so the
            # in-order PE queue interleaves it into the exp-bound stream.
            pending = []

            def drain_fillers(n=1):
                for _ in range(n):
                    if pending:
                        pending.pop(0)()

            def emit_qk_halves(which, p, sb, defer):
                xsb, wsb, bsb, dst = {
                    "q": (xq_sb, wq_sb, bq_sb, qt_sb),
                    "k": (xk_sb, wk_sb, bk_sb, kt_sb),
                }[which]
                state = {}

                def half(h):
                    def go():
                        if h == 0:
                            state["ps"] = pspj.tile([128, 512], F32, tag="pj", name="psqk")
                        ps = state["ps"]
                        for c in range(4 * h, 4 * h + 4):
                            nc.tensor.matmul(
                                ps[:],
                                wsb[:, c, 128 * p : 128 * (p + 1)],
                                xsb[:, c, 512 * sb : 512 * (sb + 1)],
                                start=(c == 0),
                                stop=(c == DC - 1),
                            )
                        if h == 1:
                            nc.vector.tensor_scalar_add(
                                dst[p][:, 512 * sb : 512 * (sb + 1)],
                                ps[:],
                                bsb[:, p : p + 1],
                            )
                    return go

                if defer:
                    pending.append(half(0))
                    pending.append(half(1))
                else:
                    half(0)()
                    half(1)()

            def emit_v_group(sc):
                # v projection, all 4 heads in one pass (N=256, 8 matmuls)
                psv = pspj.tile([128, HL * DK], F32, tag="pj", name="psv")
                for c in range(DC):
                    nc.tensor.matmul(
                        psv[:],
                        xv_sb[:, c, 128 * sc : 128 * (sc + 1)],
                        wv_sb[:, c, :],
                        start=(c == 0),
                        stop=(c == DC - 1),
                    )
                for h in range(HL):
                    nc.vector.tensor_add(
                        v_sb[:, sc, h * VW : h * VW + DK],
                        psv[:, h * DK : (h + 1) * DK],
                        bv_sb[:, h * DK : (h + 1) * DK],
                    )

            # ---- collective staging ----
            cc_in = [dram.tile([8 * 2 * DK, SQ], BF16, name=f"cc_in{p}") for p in range(2)]
            cc_out = [dram.tile([8 * 2 * DK, SQ], BF16, name=f"cc_out{p}") for p in range(2)]

            def emit_a2a(p):
                nc.gpsimd.collective_compute(
                    "AllToAll",
                    mybir.AluOpType.bypass,
                    replica_groups=[[0, 1, 2, 3, 4, 5, 6, 7]],
                    ins=[cc_in[p].opt()],
                    outs=[cc_out[p].opt()],
                )

            last_ot = [None]
            junk_tile = [None]

            def emit_junk(n):
                if not WARM_PAD:
                    return

            gate_inst = [None]

            def emit_attention_sqb(p, sqb, fill_every=0, record_gate=False):
                qsl = slice(512 * sqb, 512 * (sqb + 1))
                po = psacc.tile([128, 1024], F32, tag="acc", name="po")
                for skc in range(SKC):
                    ps2 = psmm.tile([128, 1024], F32, tag="mm", name="ps2")
                    for ch in range(2):
                        cs = slice(64 * ch, 64 * (ch + 1))
                        mm = nc.tensor.matmul(
                            ps2[:, 512 * ch : 512 * (ch + 1)],
                            kt_sb[p][cs, 128 * skc : 128 * (skc + 1)],
                            qt_sb[p][cs, qsl],
                            start=True,
                            stop=True,
                        )
                        if record_gate and skc == 0 and ch == 0:
                            gate_inst[0] = mm.ins
                    et = work.tile([128, 1024], BF16, tag="et", name="et")
                    nc.scalar.activation(
                        et[:],
                        ps2[:],
                        mybir.ActivationFunctionType.Exp,
                        bias=0.0,
                        scale=float(1.0 / np.sqrt(DK)),
                    )
                    filled = False
                    if fill_every and pending and skc % fill_every == 0:
                        drain_fillers(1)
                        filled = True
                    emit_junk(1 if filled else 2)
                    for ch in range(2):
                        h = 2 * p + ch
                        nc.tensor.matmul(
                            po[:, 512 * ch : 512 * (ch + 1)],
                            v_sb[:, skc, h * VW : h * VW + VW],
                            et[:, 512 * ch : 512 * (ch + 1)],
                            start=(skc == 0),
                            stop=(skc == SKC - 1),
                        )
                # epilogue: rowsum reciprocal, normalize, stage for the a2a
                rcp = wrk2.tile([128, 1024], F32, tag="rcp", name="rcp")
                rlo = wrk2.tile([64, 1024], F32, tag="rlo", name="rlo")
                ot = wrk2.tile([64, 1024], BF16, tag="ot", name="ot")
                nc.vector.reciprocal_approx_fast(out=rcp[:], in_=po[:])
                nc.sync.dma_start(rlo[:], rcp[64:128, :])
                nc.vector.tensor_mul(ot[:], po[0:DK, :], rlo[:])
                last_ot[0] = ot
                for shard in (sqb, sqb + 4):
                    for ch in range(2):
                        base = shard * 2 * DK + ch * DK
                        nc.sync.dma_start(
                            cc_in[p][base : base + DK, :],
                            ot[:, 512 * ch : 512 * (ch + 1)],
                        )

            # ---- schedule ----
            for h in range(HL):
                nc.vector.memset(v_sb[:, :, h * VW + DK : (h + 1) * VW], 1.0)
            # pair-0 prerequisites, tracking the xv -> xk -> xq load order
            for sc in range(SKC):
                emit_v_group(sc)
            for sb in range(4):
                emit_qk_halves("k", 0, sb, defer=False)
            emit_qk_halves("q", 0, 0, defer=False)
            # remaining projections become filler inside attention pair 0
            for sb in range(1, 4):
                emit_qk_halves("q", 0, sb, defer=True)
            for sb in range(4):
                emit_qk_halves("k", 1, sb, defer=True)
            for sb in range(4):
                emit_qk_halves("q", 1, sb, defer=True)

            for sqb in range(4):
                emit_attention_sqb(0, sqb, fill_every=3)
            drain_fillers(len(pending))
            emit_a2a(0)
            # wo reuses xq's slot (dead after qt proj), ol reuses xk's slot
            wo_sb = pers.tile([128, 2 * NR, DIN], BF16, tag="big", bufs=3, name="wo_sb")
            nc.gpsimd.dma_start(wo_sb[:], wo.rearrange("(c p) d -> p c d", p=128))
            ol_sb = pers.tile([128, 2 * NR, SQ], BF16, tag="big", bufs=3, name="ol_sb")
            # load only this batch's half of cc_out (senders 4b..4b+3) via a
            # dynamic source offset: batch = partition_id // 4
            pid = nc.gpsimd.partition_id()
            boff = (pid // 4) * (512 * SQ)

            def emit_ol(p):
                half0 = cc_out[p][0:512, :].rearrange("(c p) s -> p c s", p=128)
                src = bass.AP(
                    tensor=half0.tensor,
                    offset=half0.offset + boff,
                    ap=half0.ap,
                    dep_tracking_offset=0,
                )
                nc.gpsimd.dma_start(ol_sb[:, NR * p : NR * (p + 1), :], src)

            emit_ol(0)

            # pair-0 half of the output projection runs as filler inside the
            # last attention-pair-1 quarters (pso over chunks 0..3 -> SBUF,
            # bias pre-added so the tail is one tensor_add per tile)
            part_sb = pers.tile([128, 8, 512], F32, name="part_sb")

            def emit_opart(sb2, do):
                def go():
                    pso = pspj.tile([128, 512], F32, tag="pj", name="psop")
                    first = True
                    for c in range(NR):
                        mm = nc.tensor.matmul(
                            pso[:],
                            ol_sb[:, c, 128 * sb2 : 128 * (sb2 + 1)],
                            wo_sb[:, c, 512 * do : 512 * (do + 1)],
                            start=(c == 0),
                            stop=(c == NR - 1),
                        )
                        if first and gate_inst[0] is not None:
                            # pin the pair-0 output-projection partials behind
                            # the last attention quarter so the scheduler can't
                            # hoist them ahead of the AllToAll's completion
                            add_dep_helper(
                                mm.ins, gate_inst[0], sync=True, reason="opart gate"
                            )
                            first = False
                    nc.vector.tensor_add(
                        part_sb[:, 2 * sb2 + do, :],
                        pso[:],
                        bo_sb[:, 512 * do : 512 * (do + 1)],
                    )
                return go

            emit_attention_sqb(1, 0)
            emit_attention_sqb(1, 1)
            emit_attention_sqb(1, 2)
            for sb2 in range(4):
                for do in range(2):
                    pending.append(emit_opart(sb2, do))
            emit_attention_sqb(1, 3, fill_every=2, record_gate=True)
            drain_fillers(len(pending))
            emit_a2a(1)
            emit_ol(1)

            # warm-keeper matmuls: keep the PE busy (and HAM un-throttled)
            # while the second AllToAll is on the wire; results are unused.
            ot = last_ot[0]
            junk = psmm.tile([128, 512], F32, tag="mm", name="junk")
            for r in range(72):
                nc.tensor.matmul(
                    junk[:],
                    ot[:, 0:128],
                    ot[:, 0:512],
                    start=True,
                    stop=True,
                )

            # ---- output projection tail: pair-1 chunks + stored partials ----
            for sb2 in range(4):
                os_sb = wrk2.tile([128, DIN], F32, tag="os", name="os")
                for do in range(2):
                    pso = psmm.tile([128, 512], F32, tag="mm", name="pso")
                    for c in range(NR, 2 * NR):
                        nc.tensor.matmul(
                            pso[:],
                            ol_sb[:, c, 128 * sb2 : 128 * (sb2 + 1)],
                            wo_sb[:, c, 512 * do : 512 * (do + 1)],
                            start=(c == NR),
                            stop=(c == 2 * NR - 1),
                        )
                    nc.vector.tensor_add(
                        os_sb[:, 512 * do : 512 * (do + 1)],
                        pso[:],
                        part_sb[:, 2 * sb2 + do, :],
                    )
                nc.sync.dma_start(out[128 * sb2 : 128 * (sb2 + 1), :], os_sb[:])

    nc.compile()
    return nc


_NC = None


def _get_nc():
    global _NC
    if _NC is None:
        _NC = build()
    return _NC


def _pack_wo(Wo):
    """Row order matches ol_sb chunks: c = 4p + j (pair p, sender rank-in-group
    j whose head group is j); within a chunk, 64 rows per head hh."""
    out = np.zeros((H * DK, DIN), np.float32)
    for p in range(2):
        for j in range(NR):
            for hh in range(2):
                hg = 4 * j + 2 * p + hh
                dst = 512 * p + 128 * j + 64 * hh
                out[dst : dst + 64, :] = Wo[hg * 64 : (hg + 1) * 64, :]
    return out


def make_in_maps(Q, K, V, Wq, bq, Wk, bk, Wv, bv, Wo, bo):
    Q, K, V = (np.asarray(a, np.float32) for a in (Q, K, V))
    Wq, bq, Wk, bk, Wv, bv = (
        np.asarray(a, np.float32) for a in (Wq, bq, Wk, bk, Wv, bv)
    )
    Wo = np.asarray(Wo, np.float32)
    bo = np.asarray(bo, np.float32)
    wo_packed = np.ascontiguousarray(_pack_wo(Wo).astype(BF16NP))
    bo_b = np.ascontiguousarray(np.broadcast_to(bo, (128, DIN)))
    def _swz_x(M):
        # [DIN, S] -> [128*sblk + p, c*512 + t]
        return np.ascontiguousarray(
            M.reshape(DC, 128, 4, 512).transpose(2, 1, 0, 3).reshape(512, DC * 512)
        )

    def _swz_w(W2):
        # [DIN, 256] -> [p, c*256 + d]
        return np.ascontiguousarray(
            W2.reshape(DC, 128, HL * DK).transpose(1, 0, 2).reshape(128, DC * HL * DK)
        )

    xts = []
    for b in range(B):
        xts.append(
            tuple(_swz_x(A[b].T.astype(BF16NP)) for A in (Q, K, V))
        )
    in_maps = []
    for c in range(NCORES):
        b, g = divmod(c, 4)
        hs = slice(HL * g, HL * (g + 1))
        bq2 = np.ascontiguousarray(bq[hs].reshape(2, 128).T)
        bk2 = np.ascontiguousarray(bk[hs].reshape(2, 128).T)
        xq_t, xk_t, xv_t = xts[b]
        in_maps.append(
            {
                "xqt": xq_t,
                "xkt": xk_t,
                "xvt": xv_t,
                "wq": _swz_w(
                    Wq[hs].transpose(1, 0, 2).reshape(DIN, HL * DK).astype(BF16NP)
                ),
                "wk": _swz_w(
                    Wk[hs].transpose(1, 0, 2).reshape(DIN, HL * DK).astype(BF16NP)
                ),
                "wv": _swz_w(
                    Wv[hs].transpose(1, 0, 2).reshape(DIN, HL * DK).astype(BF16NP)
                ),
                "wo": wo_packed,
                "bqp": bq2,
                "bkp": bk2,
                "bvr": np.ascontiguousarray(
                    np.broadcast_to(bv[hs].reshape(-1), (128, HL * DK))
                ),
                "bor": bo_b,
            }
        )
    return in_maps


def run(nc, in_maps, **kwargs):
    return bass_utils.run_bass_kernel_spmd(
        nc, in_maps, core_ids=list(range(NCORES)), **kwargs
    )


def kernel(Q, K, V, Wq, bq, Wk, bk, Wv, bv, Wo, bo):
    nc = _get_nc()
    in_maps = make_in_maps(Q, K, V, Wq, bq, Wk, bk, Wv, bv, Wo, bo)
    res = run(nc, in_maps)
    full = np.empty((B, S, DIN), np.float32)
    for c in range(NCORES):
        b, g = divmod(c, 4)
        full[b, SQ * g : SQ * (g + 1), :] = res.results[c]["out"]
    return full
